# revision 1
# baseline (speedup 1.0000x reference)
"""Trainium2 Bass kernel for a pre-norm transformer block (B=4, L=2048, D=1024,
H=16, hd=64, F=4096, causal attention with additive rel-pos bias).

Sharding: 8 cores, zero collectives. Core c -> batch b = c//2, parity p = c%2.
Each core processes 8 query blocks (128 rows each) of its batch: slot j holds
q-block i = 2j + p, padded to a uniform compiled key-extent K_j = 256*(j+1)
(causal masking is folded into the bias input on the host). K/V are computed
for the full batch on both cores of a pair (redundant compute beats an
all-reduce here). All matmuls run in bf16 with fp32 PSUM accumulation;
residuals/softmax stats stay fp32.

Layouts: activations flow feature-major ("T" = [feat, row]); scores are
computed directly in [key, query] orientation so softmax needs no transposes:
Z (the softmax denominator) comes from an extra all-ones column appended to V.
"""

import sys

sys.path.insert(0, "/opt/trn_rl_repo")

import numpy as np
import ml_dtypes

import concourse.bass as bass
import concourse.mybir as mybir
import concourse.tile as tile
from concourse import bacc
from concourse.bass_utils import run_bass_kernel_spmd
from concourse.masks import make_identity

BF = ml_dtypes.bfloat16
FP32 = mybir.dt.float32
BF16 = mybir.dt.bfloat16
AF = mybir.ActivationFunctionType
ALU = mybir.AluOpType

B, L, D, H, HD, F = 4, 2048, 1024, 16, 64, 4096
NSLOT = 8                       # q-blocks per core
KENDS = [256 * (j + 1) for j in range(NSLOT)]   # compiled key extent per slot
KOFF = [16 * 128 * sum(KENDS[:j]) for j in range(NSLOT)]  # flat bias offsets
BIAS_TOT = 16 * 128 * sum(KENDS)
MASK_VAL = -30000.0


def build_nc(reps=1):
    nc = bacc.Bacc(None, target_bir_lowering=False)

    xt = nc.dram_tensor("xt", [D, L], BF16, kind="ExternalInput")
    xq = nc.dram_tensor("xq", [D, 1024], BF16, kind="ExternalInput")
    xres = nc.dram_tensor("xres", [D, 1024], FP32, kind="ExternalInput")
    biast = nc.dram_tensor("biast", [BIAS_TOT], BF16, kind="ExternalInput")
    wqt = nc.dram_tensor("wqt", [D, D], BF16, kind="ExternalInput")
    wkt = nc.dram_tensor("wkt", [D, D], BF16, kind="ExternalInput")
    wvt = nc.dram_tensor("wvt", [D, D], BF16, kind="ExternalInput")
    wot = nc.dram_tensor("wot", [D, D], BF16, kind="ExternalInput")
    w1t = nc.dram_tensor("w1t", [D, F], BF16, kind="ExternalInput")
    w2t = nc.dram_tensor("w2t", [F, D], BF16, kind="ExternalInput")
    bq = nc.dram_tensor("bq", [D], FP32, kind="ExternalInput")
    bk = nc.dram_tensor("bk", [D], FP32, kind="ExternalInput")
    bv = nc.dram_tensor("bv", [D], FP32, kind="ExternalInput")
    bo = nc.dram_tensor("bo", [D], FP32, kind="ExternalInput")
    b1 = nc.dram_tensor("b1", [F], FP32, kind="ExternalInput")
    b2 = nc.dram_tensor("b2", [D], FP32, kind="ExternalInput")
    n1w = nc.dram_tensor("n1w", [D], FP32, kind="ExternalInput")
    n2w = nc.dram_tensor("n2w", [D], FP32, kind="ExternalInput")
    outp = nc.dram_tensor("outp", [1024, D], FP32, kind="ExternalOutput")

    def emit(tc, sfx):
        pconst = tc.alloc_tile_pool(name="pconst" + sfx, bufs=1, side="left")
        ones1 = pconst.tile([128, 1], BF16)
        nc.vector.memset(ones1[:], 1.0)
        onesc = pconst.tile([1, 128], FP32)
        nc.vector.memset(onesc[:], 1.0)
        ident = pconst.tile([128, 128], FP32)
        make_identity(nc, ident[:])
        # per-partition bias/weight vectors, feature chunks on free axis
        bqt = pconst.tile([128, 8], FP32)
        nc.sync.dma_start(bqt[:], bq.rearrange("(g p) -> p g", p=128))
        bkt = pconst.tile([128, 8], FP32)
        nc.sync.dma_start(bkt[:], bk.rearrange("(g p) -> p g", p=128))
        bot = pconst.tile([128, 8], FP32)
        nc.sync.dma_start(bot[:], bo.rearrange("(g p) -> p g", p=128))
        b1t = pconst.tile([128, 32], FP32)
        nc.sync.dma_start(b1t[:], b1.rearrange("(g p) -> p g", p=128))
        b2t = pconst.tile([128, 8], FP32)
        nc.sync.dma_start(b2t[:], b2.rearrange("(g p) -> p g", p=128))
        n1t = pconst.tile([128, 8], FP32)
        nc.sync.dma_start(n1t[:], n1w.rearrange("(g p) -> p g", p=128))
        n2t = pconst.tile([128, 8], FP32)
        nc.sync.dma_start(n2t[:], n2w.rearrange("(g p) -> p g", p=128))
        epsb = pconst.tile([1, 1], FP32)
        nc.vector.memset(epsb[:], 1e-6)
        bvrow = pconst.tile([1, 1024], FP32)
        nc.sync.dma_start(bvrow[:], bv[None, :])
        # broadcast bv across partitions via ones-matmul
        bvB = pconst.tile([128, 1024], FP32)

        def bcast_row(ppool, psum_tag, src_row, dst, n):
            """dst[128, n] = broadcast of src_row[1, n] via ones-matmul."""
            for s in range(0, n, 512):
                w = min(512, n - s)
                pb = ppool.tile([128, 512], FP32, tag=psum_tag)
                nc.tensor.matmul(pb[:, :w], onesc[:], src_row[:, s:s + w],
                                 start=True, stop=True)
                nc.scalar.activation(dst[:, s:s + w], pb[:, :w], AF.Copy)

        with tc.tile_pool(name="ppbc" + sfx, bufs=2, space="PSUM") as ppbc:
            bcast_row(ppbc, "bc", bvrow, bvB, 1024)

        # ---------------- Phase 1: rmsnorm(x) for full batch and q-cols ----
        pht = tc.alloc_tile_pool(name="pht" + sfx, bufs=1, side="right")
        ht = pht.tile([128, 8, L], BF16)     # rmsnorm(x)^T, full batch
        hq = pht.tile([128, 8, 1024], BF16)  # rmsnorm(x)^T, q-cols only

        def norm_pipeline(xin, ncols, nwt, hout, pin, pnrm, pps, tag):
            # squares -> column ssq via ones-matmul -> rstd -> broadcast -> scale
            ssq = pps.tile([1, ncols], FP32, tag=tag + "ssq")
            for c in range(8):
                xs_f = pin.tile([128, 2048], BF16, tag="xs", name="xs")
                xs = xs_f[:, :ncols]
                nc.sync.dma_start(xs[:], xin[c * 128:(c + 1) * 128, :])
                sq_f = pin.tile([128, 2048], BF16, tag="sq", name="sq")
                sq = sq_f[:, :ncols]
                nc.scalar.activation(sq[:], xs[:], AF.Square)
                for s in range(0, ncols, 512):
                    nc.tensor.matmul(ssq[:, s:s + 512], ones1[:],
                                     sq[:, s:s + 512],
                                     start=(c == 0), stop=(c == 7))
            rms_f = pnrm.tile([1, 2048], FP32, tag="rms", name="rms")
            rms = rms_f[:, :ncols]
            nc.scalar.activation(rms[:], ssq[:], AF.Sqrt,
                                 scale=1.0 / D, bias=epsb[:])
            rinv_f = pnrm.tile([1, 2048], FP32, tag="rinv", name="rinv")
            rinv = rinv_f[:, :ncols]
            nc.vector.reciprocal(rinv[:], rms[:])
            rB_f = pnrm.tile([128, 2048], FP32, tag="rB", name="rB")
            rB = rB_f[:, :ncols]
            bcast_row(pps, tag + "bc", rinv, rB, ncols)
            for c in range(8):
                xs2_f = pin.tile([128, 2048], BF16, tag="xs2", name="xs2")
                xs2 = xs2_f[:, :ncols]
                nc.sync.dma_start(xs2[:], xin[c * 128:(c + 1) * 128, :])
                nc.vector.scalar_tensor_tensor(
                    hout[:, c, :], xs2[:], nwt[:, c:c + 1], rB[:],
                    op0=ALU.mult, op1=ALU.mult)

        with tc.tile_pool(name="pin1" + sfx, bufs=3) as pin1, \
             tc.tile_pool(name="pnrm1" + sfx, bufs=1) as pnrm1, \
             tc.tile_pool(name="pps1" + sfx, bufs=1, space="PSUM") as pps1:
            norm_pipeline(xt, L, n1t, ht, pin1, pnrm1, pps1, "n1")
            norm_pipeline(xq, 1024, n1t, hq, pin1, pnrm1, pps1, "nq")

        # ---------------- Phase 2: Q, K (feature-major), V (row-major) -----
        pkv = tc.alloc_tile_pool(name="pkv" + sfx, bufs=1, side="left")
        kt = pkv.tile([128, 8, L], BF16)           # K^T [feat, key]
        qt = pkv.tile([128, 8, 1024], BF16)        # Q^T [feat, query]
        vv = pkv.tile([128, 16, 16, 65], BF16)     # V rows [key, (h, hd+1)]
        nc.vector.memset(vv[:, :, :, 64:65], 1.0)

        with tc.tile_pool(name="pw2" + sfx, bufs=3) as pw2, \
             tc.tile_pool(name="pps2" + sfx, bufs=4, space="PSUM") as pps2:
            for g in range(8):
                wkg = pw2.tile([128, 8, 128], BF16, tag="wkg")
                nc.sync.dma_start(
                    wkg[:], wkt[:, g * 128:(g + 1) * 128]
                    .rearrange("(c p) o -> p c o", p=128))
                for s in range(0, L, 512):
                    pk = pps2.tile([128, 512], FP32, tag="pp", name="pk" + sfx)
                    for c in range(8):
                        nc.tensor.matmul(pk[:], wkg[:, c], ht[:, c, s:s + 512],
                                         start=(c == 0), stop=(c == 7))
                    nc.scalar.activation(kt[:, g, s:s + 512], pk[:],
                                         AF.Identity, bias=bkt[:, g:g + 1])
                wqg = pw2.tile([128, 8, 128], BF16, tag="wqg")
                nc.sync.dma_start(
                    wqg[:], wqt[:, g * 128:(g + 1) * 128]
                    .rearrange("(c p) o -> p c o", p=128))
                for s in range(0, 1024, 512):
                    pq = pps2.tile([128, 512], FP32, tag="pp", name="pq" + sfx)
                    for c in range(8):
                        nc.tensor.matmul(pq[:], wqg[:, c], hq[:, c, s:s + 512],
                                         start=(c == 0), stop=(c == 7))
                    nc.scalar.activation(qt[:, g, s:s + 512], pq[:],
                                         AF.Identity, bias=bqt[:, g:g + 1])
            wvs = pkv.tile([128, 8, 1024], BF16, tag="wvs")
            for c in range(8):
                nc.sync.dma_start(
                    wvs[:, c], wvt[c * 128:(c + 1) * 128, :])
            for lt in range(16):
                for hf in range(2):
                    pv = pps2.tile([128, 512], FP32, tag="pp", name="pv" + sfx)
                    for c in range(8):
                        nc.tensor.matmul(
                            pv[:], ht[:, c, lt * 128:(lt + 1) * 128],
                            wvs[:, c, hf * 512:(hf + 1) * 512],
                            start=(c == 0), stop=(c == 7))
                    nc.vector.tensor_add(
                        vv[:, lt, hf * 8:(hf + 1) * 8, 0:64],
                        pv[:].rearrange("p (h e) -> p h e", e=64),
                        bvB[:, hf * 512:(hf + 1) * 512]
                        .rearrange("p (h e) -> p h e", e=64))

        pht.release()

        # ---------------- Phase 3: attention ------------------------------
        pao = tc.alloc_tile_pool(name="pao" + sfx, bufs=1, side="right")
        aoT = pao.tile([128, 8, 1024], BF16)   # attn-out^T [feat, query]

        with tc.tile_pool(name="pat" + sfx, bufs=4) as pat, \
             tc.tile_pool(name="patz" + sfx, bufs=1) as patz, \
             tc.tile_pool(name="pat3" + sfx, bufs=8) as pat3, \
             tc.tile_pool(name="pps3" + sfx, bufs=1, space="PSUM") as pps3z, \
             tc.tile_pool(name="pps3s" + sfx, bufs=4, space="PSUM") as pps3s, \
             tc.tile_pool(name="pps3o" + sfx, bufs=3, space="PSUM") as pps3o:
            zB = patz.tile([128, 2048], FP32)
            for j in range(NSLOT):
                KT = KENDS[j]
                ktn = KT // 128
                zrow = patz.tile([1, 2048], FP32, tag="zrow")
                for h in range(H):
                    hg, hp = h // 2, 64 * (h % 2)
                    bias_t = pat.tile([128, 16, 128], BF16, tag="bias")
                    off = KOFF[j] + h * 128 * KT
                    nc.sync.dma_start(
                        bias_t[:, :ktn],
                        biast[off:off + 128 * KT]
                        .rearrange("(p t q) -> p t q", p=128, q=128))
                    pout = pps3o.tile([65, 128], FP32, tag="pout")
                    for kti in range(ktn):
                        ps = pps3s.tile([128, 128], FP32, tag="ps")
                        nc.tensor.matmul(
                            ps[:],
                            kt[hp:hp + 64, hg, kti * 128:(kti + 1) * 128],
                            qt[hp:hp + 64, hg, j * 128:(j + 1) * 128],
                            start=True, stop=True)
                        sc = pat3.tile([128, 128], FP32, tag="sc")
                        nc.vector.scalar_tensor_tensor(
                            sc[:], ps[:], 0.125, bias_t[:, kti],
                            op0=ALU.mult, op1=ALU.add)
                        pr = pat3.tile([128, 128], BF16, tag="pr")
                        nc.scalar.activation(pr[:], sc[:], AF.Exp)
                        nc.tensor.matmul(pout[:], vv[:, kti, h], pr[:],
                                         start=(kti == 0), stop=(kti == ktn - 1))
                    # unnormalized attn-out (rows 0..63) and Z (row 64)
                    sc65 = pat3.tile([64, 128], BF16, tag="sc65")
                    nc.scalar.activation(sc65[:], pout[0:64, :], AF.Copy)
                    zsc = pat3.tile([65, 128], FP32, tag="zsc")
                    nc.scalar.activation(zsc[64:65, :], pout[64:65, :], AF.Copy)
                    nc.sync.dma_start(zrow[0:1, h * 128:(h + 1) * 128],
                                      zsc[64:65, :])
                    nc.sync.dma_start(
                        aoT[hp:hp + 64, hg, j * 128:(j + 1) * 128], sc65[:])
                zrec = patz.tile([1, 2048], FP32, tag="zrec")
                nc.vector.reciprocal(zrec[:], zrow[:])
                bcast_row(pps3z, "zb", zrec, zB, 2048)
                for h in range(H):
                    hg, hp = h // 2, 64 * (h % 2)
                    sl = aoT[hp:hp + 64, hg, j * 128:(j + 1) * 128]
                    nc.vector.tensor_mul(
                        sl, sl, zB[hp:hp + 64, h * 128:(h + 1) * 128])

        pkv.release()

        # ---------------- Phase 4: W_O + residual --------------------------
        pres = tc.alloc_tile_pool(name="pres" + sfx, bufs=1, side="left")
        x2T = pres.tile([128, 8, 1024], FP32)
        with tc.tile_pool(name="pxr" + sfx, bufs=1) as pxr, \
             tc.tile_pool(name="pw4" + sfx, bufs=3) as pw4, \
             tc.tile_pool(name="pps4" + sfx, bufs=4, space="PSUM") as pps4:
            xr = pxr.tile([128, 8, 1024], FP32)
            for c in range(8):
                nc.sync.dma_start(xr[:, c], xres[c * 128:(c + 1) * 128, :])
            for g in range(8):
                wog = pw4.tile([128, 8, 128], BF16, tag="wog")
                nc.sync.dma_start(
                    wog[:], wot[:, g * 128:(g + 1) * 128]
                    .rearrange("(c p) o -> p c o", p=128))
                for s in range(0, 1024, 512):
                    po = pps4.tile([128, 512], FP32, tag="po")
                    for c in range(8):
                        nc.tensor.matmul(po[:], wog[:, c], aoT[:, c, s:s + 512],
                                         start=(c == 0), stop=(c == 7))
                    nc.vector.scalar_tensor_tensor(
                        x2T[:, g, s:s + 512], po[:], bot[:, g:g + 1],
                        xr[:, g, s:s + 512], op0=ALU.add, op1=ALU.add)

        pao.release()

        # ---------------- Phase 5+6: norm2 + FFN1 --------------------------
        pff = tc.alloc_tile_pool(name="pff" + sfx, bufs=1, side="right")
        f1T = pff.tile([128, 32, 1024], BF16)
        with tc.tile_pool(name="ph2" + sfx, bufs=1) as ph2, \
             tc.tile_pool(name="pn2" + sfx, bufs=3) as pn2, \
             tc.tile_pool(name="pn2s" + sfx, bufs=1) as pn2s, \
             tc.tile_pool(name="pw6" + sfx, bufs=3) as pw6, \
             tc.tile_pool(name="pps6" + sfx, bufs=4, space="PSUM") as pps6, \
             tc.tile_pool(name="pps6b" + sfx, bufs=2, space="PSUM") as pps6b, \
             tc.tile_pool(name="pps6s" + sfx, bufs=1, space="PSUM") as pps6s:
            h2 = ph2.tile([128, 8, 1024], BF16)
            ssq = pps6s.tile([1, 1024], FP32)
            for c in range(8):
                sq = pn2.tile([128, 1024], BF16, tag="sq2")
                nc.scalar.activation(sq[:], x2T[:, c, :], AF.Square)
                for s in range(0, 1024, 512):
                    nc.tensor.matmul(ssq[:, s:s + 512], ones1[:],
                                     sq[:, s:s + 512],
                                     start=(c == 0), stop=(c == 7))
            rms = pn2s.tile([1, 1024], FP32, tag="rms2")
            nc.scalar.activation(rms[:], ssq[:], AF.Sqrt,
                                 scale=1.0 / D, bias=epsb[:])
            rinv = pn2s.tile([1, 1024], FP32, tag="rinv2")
            nc.vector.reciprocal(rinv[:], rms[:])
            rB = pn2s.tile([128, 1024], FP32, tag="rB2")
            bcast_row(pps6b, "bc2", rinv, rB, 1024)
            for c in range(8):
                nc.vector.scalar_tensor_tensor(
                    h2[:, c, :], x2T[:, c, :], n2t[:, c:c + 1], rB[:],
                    op0=ALU.mult, op1=ALU.mult)
            for gf in range(32):
                w1g = pw6.tile([128, 8, 128], BF16, tag="w1g")
                nc.sync.dma_start(
                    w1g[:], w1t[:, gf * 128:(gf + 1) * 128]
                    .rearrange("(c p) o -> p c o", p=128))
                for s in range(0, 1024, 512):
                    p1 = pps6.tile([128, 512], FP32, tag="p1")
                    for c in range(8):
                        nc.tensor.matmul(p1[:], w1g[:, c], h2[:, c, s:s + 512],
                                         start=(c == 0), stop=(c == 7))
                    nc.scalar.activation(f1T[:, gf, s:s + 512], p1[:],
                                         AF.Silu, bias=b1t[:, gf:gf + 1])

        # ---------------- Phase 7: FFN2 + residual -------------------------
        pout_pool = tc.alloc_tile_pool(name="poutp" + sfx, bufs=1, side="left")
        oT = pout_pool.tile([128, 8, 1024], FP32)
        with tc.tile_pool(name="pw7" + sfx, bufs=3) as pw7, \
             tc.tile_pool(name="pps7" + sfx, bufs=4, space="PSUM") as pps7:
            for g in range(8):
                w2g = pw7.tile([128, 32, 128], BF16, tag="w2g")
                nc.sync.dma_start(
                    w2g[:], w2t[:, g * 128:(g + 1) * 128]
                    .rearrange("(c p) o -> p c o", p=128))
                for s in range(0, 1024, 512):
                    p2 = pps7.tile([128, 512], FP32, tag="p2")
                    for cf in range(32):
                        nc.tensor.matmul(p2[:], w2g[:, cf], f1T[:, cf, s:s + 512],
                                         start=(cf == 0), stop=(cf == 31))
                    nc.vector.scalar_tensor_tensor(
                        oT[:, g, s:s + 512], p2[:], b2t[:, g:g + 1],
                        x2T[:, g, s:s + 512], op0=ALU.add, op1=ALU.add)

        pff.release()

        # ---------------- Phase 8: transpose to row-major + store ----------
        with tc.tile_pool(name="po8" + sfx, bufs=4) as po8, \
             tc.tile_pool(name="pps8" + sfx, bufs=4, space="PSUM") as pps8:
            for j in range(NSLOT):
                for g in range(8):
                    pt = pps8.tile([128, 128], FP32, tag="pt")
                    nc.tensor.transpose(pt[:], oT[:, g, j * 128:(j + 1) * 128],
                                        ident[:])
                    ot = po8.tile([128, 128], FP32, tag="ot")
                    nc.scalar.activation(ot[:], pt[:], AF.Copy)
                    nc.sync.dma_start(
                        outp[j * 128:(j + 1) * 128, g * 128:(g + 1) * 128],
                        ot[:])

        pout_pool.release()
        pres.release()
        pconst.release()

    with tile.TileContext(nc, pool_alloc_mode="queue") as tc:
        for r in range(reps):
            emit(tc, str(r) if reps > 1 else "")

    nc.compile()
    return nc


_NC = None


def _get_nc():
    global _NC
    if _NC is None:
        _NC = build_nc()
    return _NC


def _prep_core_inputs(c, x, rel_pos_bias, wq, bq, wk, bk, wv, bv, wo, bo,
                      norm1_w, norm2_w, w1, b1, w2, b2):
    b, par = c // 2, c % 2
    xb = x[b]                                   # [L, D] f32
    qrows = np.concatenate(
        [np.arange(128 * (2 * j + par), 128 * (2 * j + par) + 128)
         for j in range(NSLOT)])
    bias_flat = np.empty(BIAS_TOT, dtype=BF)
    kidx = np.arange(L)
    for j in range(NSLOT):
        i = 2 * j + par
        KT = KENDS[j]
        q0 = 128 * i
        blk = rel_pos_bias[0, :, q0:q0 + 128, :KT].astype(np.float32)
        mask = kidx[None, :KT] > (q0 + np.arange(128))[:, None]
        blk = np.where(mask[None], MASK_VAL, blk)          # [16, 128q, KT]
        # -> [16, 128p(k%128), kt, 128q] flattened
        t = blk.transpose(0, 2, 1).reshape(16, KT // 128, 128, 128)
        t = t.transpose(0, 2, 1, 3)
        bias_flat[KOFF[j]:KOFF[j] + 16 * 128 * KT] = \
            t.reshape(-1).astype(BF)
    return {
        "xt": np.ascontiguousarray(xb.T).astype(BF),
        "xq": np.ascontiguousarray(xb[qrows].T).astype(BF),
        "xres": np.ascontiguousarray(xb[qrows].T).astype(np.float32),
        "biast": bias_flat,
        "wqt": np.ascontiguousarray(wq.T).astype(BF),
        "wkt": np.ascontiguousarray(wk.T).astype(BF),
        "wvt": np.ascontiguousarray(wv.T).astype(BF),
        "wot": np.ascontiguousarray(wo.T).astype(BF),
        "w1t": np.ascontiguousarray(w1.T).astype(BF),
        "w2t": np.ascontiguousarray(w2.T).astype(BF),
        "bq": bq.astype(np.float32), "bk": bk.astype(np.float32),
        "bv": bv.astype(np.float32), "bo": bo.astype(np.float32),
        "b1": b1.astype(np.float32), "b2": b2.astype(np.float32),
        "n1w": norm1_w.astype(np.float32), "n2w": norm2_w.astype(np.float32),
    }


def kernel(**inputs):
    inputs = {k: np.asarray(v) for k, v in inputs.items()}
    nc = _get_nc()
    in_maps = [_prep_core_inputs(c, **inputs) for c in range(8)]
    res = run_bass_kernel_spmd(nc, in_maps, core_ids=list(range(8)))
    out = np.empty((B, L, D), dtype=np.float32)
    for c in range(8):
        b, par = c // 2, c % 2
        o = res.results[c]["outp"]
        for j in range(NSLOT):
            i = 2 * j + par
            out[b, 128 * i:128 * i + 128] = o[128 * j:128 * j + 128]
    return out



# revision 10
# speedup vs baseline: 1.1730x; 1.1730x over previous
"""Trainium2 Bass kernel for a pre-norm transformer block (B=4, L=2048, D=1024,
H=16, hd=64, F=4096, causal attention with additive rel-pos bias).

Sharding: 8 cores, zero collectives. Core c -> batch b = c//2, parity p = c%2.
Each core processes 8 query blocks (128 rows each) of its batch, interleaved
by parity (q-block i = 2j + p for slot j), so causal load is balanced.

v2 design notes (vs v1):
- Scores are computed per (head, key-tile) with a single wide matmul over all
  query slots that need that key tile (contiguous columns [j0*128, 1024),
  j0 = kti//2), split at the 512-column PSUM bank boundary into two q-half
  passes (C0: cols 0:512 / kti 0..7, C1: cols 512:1024 / kti 0..15).
- The rel-pos bias (with causal mask folded in, host-side) is accumulated
  into the scores PSUM by an identity-weights matmul - no vector-engine
  scale/bias op in the softmax inner loop. The 1/sqrt(hd) scale is folded
  into wq/bq host-side.
- exp runs once per (head, key-tile) on the wide tile; P (exp'd scores) lands
  in persistent per-kti SBUF buffers whose causally-dead columns are zeroed
  once at startup, so PV accumulation is uniform full-width with simple
  start/stop flags.
- Z comes from an all-ones column appended to V (row 64 of the PV output);
  normalization is DVE reciprocal + gpsimd partition_broadcast + one DVE
  multiply that also casts attn-out to bf16.
- FFN SiLU is computed as x*0.5*(1+tanh(x/2)) (exact), with the 0.5 folded
  into w2 host-side, so every table-based activation in the kernel (exp,
  tanh, square, copy, identity) lives in the same activation table set and
  only the three tiny rmsnorm Sqrt ops force a table switch.
- All bulk DMAs are batched and issued from the gpsimd queue.
"""

import sys

sys.path.insert(0, "/opt/trn_rl_repo")

import numpy as np
import ml_dtypes

import concourse.bass as bass
import concourse.mybir as mybir
import concourse.tile as tile
from concourse import bacc
from concourse.bass_utils import run_bass_kernel_spmd
from concourse.masks import make_identity

BF = ml_dtypes.bfloat16
FP32 = mybir.dt.float32
BF16 = mybir.dt.bfloat16
AF = mybir.ActivationFunctionType
ALU = mybir.AluOpType

B, L, D, H, HD, F = 4, 2048, 1024, 16, 64, 4096
MASK_VAL = -30000.0

# C0 pass: q cols [0, 512), key tiles 0..7, score width W0 = 512 - j0*128
KTI0 = 8
W0 = [512 - (k // 2) * 128 for k in range(KTI0)]
CUM0 = [sum(W0[:k]) for k in range(KTI0)]
TOT0 = sum(W0)                       # 2560
# C1 pass: q cols [512, 1024), key tiles 0..15
KTI1 = 16
C1S = [max(512, (k // 2) * 128) for k in range(KTI1)]   # global col start
W1 = [1024 - c for c in C1S]
CUM1 = [sum(W1[:k]) for k in range(KTI1)]
TOT1 = sum(W1)                       # 6656
BIAS_PER_HEAD = TOT0 + TOT1          # 9216
BIAS_TOT = H * 128 * BIAS_PER_HEAD


def build_nc():
    nc = bacc.Bacc(None, target_bir_lowering=False)

    xt = nc.dram_tensor("xt", [D, L], BF16, kind="ExternalInput")
    xq = nc.dram_tensor("xq", [D, 1024], BF16, kind="ExternalInput")
    xres = nc.dram_tensor("xres", [D, 1024], FP32, kind="ExternalInput")
    biast = nc.dram_tensor("biast", [BIAS_TOT], BF16, kind="ExternalInput")
    wqt = nc.dram_tensor("wqt", [D, D], BF16, kind="ExternalInput")
    wkt = nc.dram_tensor("wkt", [D, D], BF16, kind="ExternalInput")
    wvt = nc.dram_tensor("wvt", [D, D], BF16, kind="ExternalInput")
    wot = nc.dram_tensor("wot", [D, D], BF16, kind="ExternalInput")
    w1t = nc.dram_tensor("w1t", [D, F], BF16, kind="ExternalInput")
    w2t = nc.dram_tensor("w2t", [F, D], BF16, kind="ExternalInput")
    bq = nc.dram_tensor("bq", [D], FP32, kind="ExternalInput")
    bk = nc.dram_tensor("bk", [D], FP32, kind="ExternalInput")
    bv = nc.dram_tensor("bv", [D], FP32, kind="ExternalInput")
    bo = nc.dram_tensor("bo", [D], FP32, kind="ExternalInput")
    b1h = nc.dram_tensor("b1h", [F], FP32, kind="ExternalInput")  # 0.5*b1
    b1f = nc.dram_tensor("b1f", [F], FP32, kind="ExternalInput")  # b1
    b2 = nc.dram_tensor("b2", [D], FP32, kind="ExternalInput")
    n1w = nc.dram_tensor("n1w", [D], FP32, kind="ExternalInput")
    n2w = nc.dram_tensor("n2w", [D], FP32, kind="ExternalInput")
    outp = nc.dram_tensor("outp", [1024, D], FP32, kind="ExternalOutput")

    def emit(tc):
        # ---------------- constants ------------------------------------
        pconst = tc.alloc_tile_pool(name="pconst", bufs=1, side="left")
        ones1 = pconst.tile([128, 1], BF16)
        nc.vector.memset(ones1[:], 1.0)
        identB = pconst.tile([128, 128], BF16)
        make_identity(nc, identB[:])
        identT = pconst.tile([128, 128], FP32)
        make_identity(nc, identT[:])
        bqt = pconst.tile([128, 8], FP32)
        nc.gpsimd.dma_start(bqt[:], bq.rearrange("(g p) -> p g", p=128))
        bkt = pconst.tile([128, 8], FP32)
        nc.gpsimd.dma_start(bkt[:], bk.rearrange("(g p) -> p g", p=128))
        bot = pconst.tile([128, 8], FP32)
        nc.gpsimd.dma_start(bot[:], bo.rearrange("(g p) -> p g", p=128))
        b1ht = pconst.tile([128, 32], FP32)
        nc.gpsimd.dma_start(b1ht[:], b1h.rearrange("(g p) -> p g", p=128))
        b1ft = pconst.tile([128, 32], FP32)
        nc.gpsimd.dma_start(b1ft[:], b1f.rearrange("(g p) -> p g", p=128))
        b2t = pconst.tile([128, 8], FP32)
        nc.gpsimd.dma_start(b2t[:], b2.rearrange("(g p) -> p g", p=128))
        n1t = pconst.tile([128, 8], FP32)
        nc.gpsimd.dma_start(n1t[:], n1w.rearrange("(g p) -> p g", p=128))
        n2t = pconst.tile([128, 8], FP32)
        nc.gpsimd.dma_start(n2t[:], n2w.rearrange("(g p) -> p g", p=128))
        epsb = pconst.tile([1, 1], FP32)
        nc.vector.memset(epsb[:], 1e-6)
        bvrow = pconst.tile([1, 1024], FP32)
        nc.gpsimd.dma_start(bvrow[:], bv[None, :])
        bvB = pconst.tile([128, 1024], FP32)
        nc.gpsimd.partition_broadcast(bvB[:], bvrow[:])

        # residual (q-rows, feature-major, fp32) - streams in early
        pxr = tc.alloc_tile_pool(name="pxr", bufs=1, side="left")
        xr = pxr.tile([128, 8, 1024], FP32)
        for c in range(8):
            nc.gpsimd.dma_start(xr[:, c], xres[c * 128:(c + 1) * 128, :])

        # ---------------- Phase A: norm1 -------------------------------
        pht = tc.alloc_tile_pool(name="pht", bufs=1, side="right")
        ht = pht.tile([128, 8, L], BF16)     # rmsnorm(x)^T, full batch
        hq = pht.tile([128, 8, 1024], BF16)  # rmsnorm(x)^T, q-cols only

        with tc.tile_pool(name="pxs", bufs=1) as pxs, \
             tc.tile_pool(name="psq", bufs=2) as psq, \
             tc.tile_pool(name="pnrm", bufs=1) as pnrm, \
             tc.tile_pool(name="ppsA", bufs=1, space="PSUM") as ppsA:
            xs = pxs.tile([128, 8, L], BF16)
            xqs = pxs.tile([128, 8, 1024], BF16)
            ssq = ppsA.tile([1, L], FP32)
            ssqq = ppsA.tile([1, 1024], FP32)
            for c in range(8):
                nc.gpsimd.dma_start(xs[:, c], xt[c * 128:(c + 1) * 128, :])
                sq = psq.tile([128, L], BF16, tag="sq")
                nc.scalar.activation(sq[:], xs[:, c], AF.Square)
                for s in range(0, L, 512):
                    nc.tensor.matmul(ssq[:, s:s + 512], ones1[:],
                                     sq[:, s:s + 512],
                                     start=(c == 0), stop=(c == 7))
                nc.gpsimd.dma_start(xqs[:, c], xq[c * 128:(c + 1) * 128, :])
                sqq = psq.tile([128, 1024], BF16, tag="sqq")
                nc.scalar.activation(sqq[:], xqs[:, c], AF.Square)
                for s in range(0, 1024, 512):
                    nc.tensor.matmul(ssqq[:, s:s + 512], ones1[:],
                                     sqq[:, s:s + 512],
                                     start=(c == 0), stop=(c == 7))
            rms = pnrm.tile([1, L], FP32)
            nc.scalar.activation(rms[:], ssq[:], AF.Sqrt,
                                 scale=1.0 / D, bias=epsb[:])
            rinv = pnrm.tile([1, L], FP32)
            nc.vector.reciprocal(rinv[:], rms[:])
            rB = pnrm.tile([128, L], FP32)
            nc.gpsimd.partition_broadcast(rB[:], rinv[:])
            rmsq = pnrm.tile([1, 1024], FP32)
            nc.scalar.activation(rmsq[:], ssqq[:], AF.Sqrt,
                                 scale=1.0 / D, bias=epsb[:])
            rinvq = pnrm.tile([1, 1024], FP32)
            nc.vector.reciprocal(rinvq[:], rmsq[:])
            rBq = pnrm.tile([128, 1024], FP32)
            nc.gpsimd.partition_broadcast(rBq[:], rinvq[:])
            for c in range(8):
                nc.vector.scalar_tensor_tensor(
                    ht[:, c, :], xs[:, c], n1t[:, c:c + 1], rB[:],
                    op0=ALU.mult, op1=ALU.mult)
                nc.vector.scalar_tensor_tensor(
                    hq[:, c, :], xqs[:, c], n1t[:, c:c + 1], rBq[:],
                    op0=ALU.mult, op1=ALU.mult)

        # ---------------- Phase B: Q, K (feature-major), V (row-major) --
        pkv = tc.alloc_tile_pool(name="pkv", bufs=1, side="left")
        kt = pkv.tile([128, 8, L], BF16)           # K^T [feat, key]
        qt = pkv.tile([128, 8, 1024], BF16)        # Q^T [feat, query]
        vv = pkv.tile([128, 16, 16, 65], BF16)     # V rows [key, (h, hd+1)]
        nc.vector.memset(vv[:, :, :, 64:65], 1.0)

        with tc.tile_pool(name="pw2", bufs=3) as pw2, \
             tc.tile_pool(name="pwv", bufs=1) as pwv, \
             tc.tile_pool(name="pps2", bufs=4, space="PSUM") as pps2:
            for g in range(8):
                wkg = pw2.tile([128, 8, 128], BF16, tag="wkg")
                nc.gpsimd.dma_start(
                    wkg[:], wkt[:, g * 128:(g + 1) * 128]
                    .rearrange("(c p) o -> p c o", p=128))
                for s in range(0, L, 512):
                    pk = pps2.tile([128, 512], FP32, tag="pp", name="pk")
                    for c in range(8):
                        nc.tensor.matmul(pk[:], wkg[:, c], ht[:, c, s:s + 512],
                                         start=(c == 0), stop=(c == 7))
                    nc.scalar.activation(kt[:, g, s:s + 512], pk[:],
                                         AF.Identity, bias=bkt[:, g:g + 1])
                wqg = pw2.tile([128, 8, 128], BF16, tag="wqg")
                nc.gpsimd.dma_start(
                    wqg[:], wqt[:, g * 128:(g + 1) * 128]
                    .rearrange("(c p) o -> p c o", p=128))
                for s in range(0, 1024, 512):
                    pq = pps2.tile([128, 512], FP32, tag="pp", name="pq")
                    for c in range(8):
                        nc.tensor.matmul(pq[:], wqg[:, c], hq[:, c, s:s + 512],
                                         start=(c == 0), stop=(c == 7))
                    nc.scalar.activation(qt[:, g, s:s + 512], pq[:],
                                         AF.Identity, bias=bqt[:, g:g + 1])
            wvs = pwv.tile([128, 8, 1024], BF16, tag="wvs")
            for c in range(8):
                nc.gpsimd.dma_start(wvs[:, c], wvt[c * 128:(c + 1) * 128, :])
            for lt in range(16):
                for hf in range(2):
                    pv = pps2.tile([128, 512], FP32, tag="pp", name="pv")
                    for c in range(8):
                        nc.tensor.matmul(
                            pv[:], ht[:, c, lt * 128:(lt + 1) * 128],
                            wvs[:, c, hf * 512:(hf + 1) * 512],
                            start=(c == 0), stop=(c == 7))
                    nc.vector.tensor_add(
                        vv[:, lt, hf * 8:(hf + 1) * 8, 0:64],
                        pv[:].rearrange("p (h e) -> p h e", e=64),
                        bvB[:, hf * 512:(hf + 1) * 512]
                        .rearrange("p (h e) -> p h e", e=64))

        pht.release()

        # ---------------- Phase C: attention ----------------------------
        pao = tc.alloc_tile_pool(name="pao", bufs=1, side="right")
        aoT = pao.tile([128, 8, 1024], BF16)   # attn-out^T [feat, query]

        def attn_pass(npass, kti_n, wlist, cumlist, bias_off):
            sfx = str(npass)
            pP = tc.alloc_tile_pool(name="pP" + sfx, bufs=1, side="right")
            P = pP.tile([128, kti_n, 512], BF16, name="P" + sfx)
            # zero causally-dead columns once (PV reads them as zeros)
            for k in range(kti_n):
                dead = 512 - wlist[k]
                if dead > 0:
                    nc.gpsimd.memset(P[:, k, 0:dead], 0.0)
            qbase = npass * 512
            tot = cumlist[-1] + wlist[-1]
            with tc.tile_pool(name="pbias" + sfx, bufs=2) as pbias, \
                 tc.tile_pool(name="pz" + sfx, bufs=2) as pz, \
                 tc.tile_pool(name="pS" + sfx, bufs=3, space="PSUM") as pS, \
                 tc.tile_pool(name="pPo" + sfx, bufs=2, space="PSUM") as pPo:
                for h in range(H):
                    hg, hp = h // 2, 64 * (h % 2)
                    bst = pbias.tile([128, tot], BF16, tag="bst",
                                     name="bst" + sfx)
                    off = bias_off + h * 128 * tot
                    nc.gpsimd.dma_start(
                        bst[:, :tot],
                        biast[off:off + 128 * tot]
                        .rearrange("(p w) -> p w", p=128))
                    pout = pPo.tile([65, 512], FP32, tag="po", name="po" + sfx)
                    for k in range(kti_n):
                        w = wlist[k]
                        dead = 512 - w
                        S = pS.tile([128, 512], FP32, tag="S", name="S" + sfx)
                        nc.tensor.matmul(
                            S[:, 0:w],
                            kt[hp:hp + 64, hg, k * 128:(k + 1) * 128],
                            qt[hp:hp + 64, hg, qbase + dead:qbase + 512],
                            start=True, stop=False)
                        nc.tensor.matmul(
                            S[:, 0:w], identB[:],
                            bst[:, cumlist[k]:cumlist[k] + w],
                            start=False, stop=True)
                        nc.scalar.activation(P[:, k, dead:512], S[:, 0:w],
                                             AF.Exp)
                        nc.tensor.matmul(pout[:], vv[:, k, h], P[:, k, :],
                                         start=(k == 0), stop=(k == kti_n - 1))
                    zrec = pz.tile([1, 512], FP32, tag="zr", name="zr" + sfx)
                    nc.vector.reciprocal(zrec[:], pout[64:65, :])
                    zbB = pz.tile([64, 512], FP32, tag="zb", name="zb" + sfx)
                    nc.gpsimd.partition_broadcast(zbB[:], zrec[:])
                    nc.vector.tensor_mul(
                        aoT[hp:hp + 64, hg, qbase:qbase + 512],
                        pout[0:64, :], zbB[:])
            pP.release()

        attn_pass(0, KTI0, W0, CUM0, 0)
        attn_pass(1, KTI1, W1, CUM1, H * 128 * TOT0)

        pkv.release()

        # ---------------- Phase D: W_O + FFN, per q-half ----------------
        pwo = tc.alloc_tile_pool(name="pwo", bufs=1, side="left")
        wov = pwo.tile([128, 8, 8, 128], BF16)   # all of wo resident
        for g in range(8):
            nc.gpsimd.dma_start(
                wov[:, g], wot[:, g * 128:(g + 1) * 128]
                .rearrange("(c p) o -> p c o", p=128))
        px2 = tc.alloc_tile_pool(name="px2", bufs=1, side="left")
        x2T = px2.tile([128, 8, 1024], FP32)

        def ffn_half(n):
            sfx = "h" + str(n)
            q0 = n * 512
            with tc.tile_pool(name="ph2" + sfx, bufs=1) as ph2, \
                 tc.tile_pool(name="pf1" + sfx, bufs=1) as pf1, \
                 tc.tile_pool(name="pwf" + sfx, bufs=3) as pwf, \
                 tc.tile_pool(name="pnrm" + sfx, bufs=1) as pnrm, \
                 tc.tile_pool(name="psq" + sfx, bufs=2) as psq, \
                 tc.tile_pool(name="pu" + sfx, bufs=3) as pu, \
                 tc.tile_pool(name="pot" + sfx, bufs=2) as pot, \
                 tc.tile_pool(name="ppsD" + sfx, bufs=4, space="PSUM") as ppsD, \
                 tc.tile_pool(name="ppsS" + sfx, bufs=1, space="PSUM") as ppsS:
                # W_O + residual
                for g in range(8):
                    po = ppsD.tile([128, 512], FP32, tag="pp", name="po" + sfx)
                    for c in range(8):
                        nc.tensor.matmul(po[:], wov[:, g, c],
                                         aoT[:, c, q0:q0 + 512],
                                         start=(c == 0), stop=(c == 7))
                    nc.vector.scalar_tensor_tensor(
                        x2T[:, g, q0:q0 + 512], po[:], bot[:, g:g + 1],
                        xr[:, g, q0:q0 + 512], op0=ALU.add, op1=ALU.add)
                # norm2
                ssq = ppsS.tile([1, 512], FP32, name="ssq" + sfx)
                for c in range(8):
                    sq = psq.tile([128, 512], BF16, tag="sq", name="sq" + sfx)
                    nc.scalar.activation(sq[:], x2T[:, c, q0:q0 + 512],
                                         AF.Square)
                    nc.tensor.matmul(ssq[:], ones1[:], sq[:],
                                     start=(c == 0), stop=(c == 7))
                rms = pnrm.tile([1, 512], FP32, name="rms" + sfx)
                nc.scalar.activation(rms[:], ssq[:], AF.Sqrt,
                                     scale=1.0 / D, bias=epsb[:])
                rinv = pnrm.tile([1, 512], FP32, name="rinv" + sfx)
                nc.vector.reciprocal(rinv[:], rms[:])
                rB = pnrm.tile([128, 512], FP32, name="rB" + sfx)
                nc.gpsimd.partition_broadcast(rB[:], rinv[:])
                h2 = ph2.tile([128, 8, 512], BF16, name="h2" + sfx)
                for c in range(8):
                    nc.vector.scalar_tensor_tensor(
                        h2[:, c, :], x2T[:, c, q0:q0 + 512], n2t[:, c:c + 1],
                        rB[:], op0=ALU.mult, op1=ALU.mult)
                # FFN1: f1 = (z + b1) * (1 + tanh(z/2 + b1/2)) = 2*silu(z+b1)
                f1T = pf1.tile([128, 32, 512], BF16, name="f1T" + sfx)
                for gf in range(32):
                    w1g = pwf.tile([128, 8, 128], BF16, tag="w1g",
                                   name="w1g" + sfx)
                    nc.gpsimd.dma_start(
                        w1g[:], w1t[:, gf * 128:(gf + 1) * 128]
                        .rearrange("(c p) o -> p c o", p=128))
                    z = ppsD.tile([128, 512], FP32, tag="pp", name="z" + sfx)
                    for c in range(8):
                        nc.tensor.matmul(z[:], w1g[:, c], h2[:, c, :],
                                         start=(c == 0), stop=(c == 7))
                    u = pu.tile([128, 512], BF16, tag="u", name="u" + sfx)
                    nc.scalar.activation(u[:], z[:], AF.Tanh,
                                         scale=0.5, bias=b1ht[:, gf:gf + 1])
                    t2 = pu.tile([128, 512], BF16, tag="t2", name="t2" + sfx)
                    nc.vector.scalar_tensor_tensor(
                        t2[:], z[:], b1ft[:, gf:gf + 1], u[:],
                        op0=ALU.add, op1=ALU.mult)
                    nc.vector.scalar_tensor_tensor(
                        f1T[:, gf, :], z[:], b1ft[:, gf:gf + 1], t2[:],
                        op0=ALU.add, op1=ALU.add)
                # FFN2 + residual + transpose + store
                for g in range(8):
                    w2g = pwf.tile([128, 32, 128], BF16, tag="w2g",
                                   name="w2g" + sfx)
                    nc.gpsimd.dma_start(
                        w2g[:], w2t[:, g * 128:(g + 1) * 128]
                        .rearrange("(c p) o -> p c o", p=128))
                    o = ppsD.tile([128, 512], FP32, tag="pp", name="o" + sfx)
                    for cf in range(32):
                        nc.tensor.matmul(o[:], w2g[:, cf], f1T[:, cf, :],
                                         start=(cf == 0), stop=(cf == 31))
                    og = pot.tile([128, 512], FP32, tag="og", name="og" + sfx)
                    nc.vector.scalar_tensor_tensor(
                        og[:], o[:], b2t[:, g:g + 1],
                        x2T[:, g, q0:q0 + 512], op0=ALU.add, op1=ALU.add)
                    for j in range(4):
                        pt = ppsD.tile([128, 128], FP32, tag="pt", bufs=2,
                                       name="pt" + sfx)
                        nc.tensor.transpose(pt[:], og[:, j * 128:(j + 1) * 128],
                                            identT[:])
                        ot = pot.tile([128, 128], FP32, tag="ot",
                                      name="ot" + sfx)
                        nc.vector.tensor_copy(ot[:], pt[:])
                        nc.gpsimd.dma_start(
                            outp[(n * 4 + j) * 128:(n * 4 + j + 1) * 128,
                                 g * 128:(g + 1) * 128],
                            ot[:])

        ffn_half(0)
        ffn_half(1)

        pao.release()
        px2.release()
        pwo.release()
        pxr.release()
        pconst.release()

    with tile.TileContext(nc, pool_alloc_mode="queue") as tc:
        emit(tc)

    nc.compile()
    return nc


_NC = None


def _get_nc():
    global _NC
    if _NC is None:
        _NC = build_nc()
    return _NC


def _prep_core_inputs(c, x, rel_pos_bias, wq, bq, wk, bk, wv, bv, wo, bo,
                      norm1_w, norm2_w, w1, b1, w2, b2):
    b, par = c // 2, c % 2
    xb = x[b]                                   # [L, D] f32
    qrows = np.concatenate(
        [np.arange(128 * (2 * j + par), 128 * (2 * j + par) + 128)
         for j in range(8)])
    rel = np.asarray(rel_pos_bias[0], dtype=np.float32)   # [H, L, L]

    bias_flat = np.empty(BIAS_TOT, dtype=BF)
    # C0 region: per head, key tiles 0..7, q cols [dead, 512) of this core
    off = 0
    for npass, (kti_n, wlist, cumlist, qbase) in enumerate(
            [(KTI0, W0, CUM0, 0), (KTI1, W1, CUM1, 512)]):
        tot = cumlist[-1] + wlist[-1]
        blk = np.empty((H, 128, tot), dtype=np.float32)
        for k in range(kti_n):
            w = wlist[k]
            dead = 512 - w
            qcols = qbase + dead + np.arange(w)           # local q col idx
            j = qcols // 128                              # slot
            i = 2 * j + par                               # q block
            qglob = i * 128 + (qcols % 128)               # global q row
            k0 = k * 128
            sub = rel[:, qglob, k0:k0 + 128]              # [H, w, 128]
            mask = (k0 + np.arange(128))[None, :] > qglob[:, None]  # [w,128]
            sub = np.where(mask[None], MASK_VAL, sub)
            blk[:, :, cumlist[k]:cumlist[k] + w] = sub.transpose(0, 2, 1)
        bias_flat[off:off + H * 128 * tot] = blk.reshape(-1).astype(BF)
        off += H * 128 * tot

    scale = HD ** -0.5
    return {
        "xt": np.ascontiguousarray(xb.T).astype(BF),
        "xq": np.ascontiguousarray(xb[qrows].T).astype(BF),
        "xres": np.ascontiguousarray(xb[qrows].T).astype(np.float32),
        "biast": bias_flat,
        "wqt": np.ascontiguousarray(wq.T * scale).astype(BF),
        "wkt": np.ascontiguousarray(wk.T).astype(BF),
        "wvt": np.ascontiguousarray(wv.T).astype(BF),
        "wot": np.ascontiguousarray(wo.T).astype(BF),
        "w1t": np.ascontiguousarray(w1.T).astype(BF),
        "w2t": np.ascontiguousarray(w2.T * 0.5).astype(BF),
        "bq": (bq * scale).astype(np.float32),
        "bk": bk.astype(np.float32),
        "bv": bv.astype(np.float32), "bo": bo.astype(np.float32),
        "b1h": (0.5 * b1).astype(np.float32),
        "b1f": b1.astype(np.float32),
        "b2": b2.astype(np.float32),
        "n1w": norm1_w.astype(np.float32), "n2w": norm2_w.astype(np.float32),
    }


def kernel(**inputs):
    inputs = {k: np.asarray(v) for k, v in inputs.items()}
    nc = _get_nc()
    in_maps = [_prep_core_inputs(c, **inputs) for c in range(8)]
    res = run_bass_kernel_spmd(nc, in_maps, core_ids=list(range(8)))
    out = np.empty((B, L, D), dtype=np.float32)
    for c in range(8):
        b, par = c // 2, c % 2
        o = res.results[c]["outp"]
        for j in range(8):
            i = 2 * j + par
            out[b, 128 * i:128 * i + 128] = o[128 * j:128 * j + 128]
    return out


# revision 13
# speedup vs baseline: 1.2627x; 1.0764x over previous
"""Trainium2 Bass kernel for a pre-norm transformer block (B=4, L=2048, D=1024,
H=16, hd=64, F=4096, causal attention with additive rel-pos bias).

Sharding: 8 cores, zero collectives. Core c -> batch b = c//2, parity p = c%2.
Each core processes 8 query blocks (128 rows each) of its batch, interleaved
by parity (q-block i = 2j + p for slot j), so causal load is balanced.

v2 design notes (vs v1):
- Scores are computed per (head, key-tile) with a single wide matmul over all
  query slots that need that key tile (contiguous columns [j0*128, 1024),
  j0 = kti//2), split at the 512-column PSUM bank boundary into two q-half
  passes (C0: cols 0:512 / kti 0..7, C1: cols 512:1024 / kti 0..15).
- The rel-pos bias (with causal mask folded in, host-side) is accumulated
  into the scores PSUM by an identity-weights matmul - no vector-engine
  scale/bias op in the softmax inner loop. The 1/sqrt(hd) scale is folded
  into wq/bq host-side.
- exp runs once per (head, key-tile) on the wide tile; P (exp'd scores) lands
  in persistent per-kti SBUF buffers whose causally-dead columns are zeroed
  once at startup, so PV accumulation is uniform full-width with simple
  start/stop flags.
- Z comes from an all-ones column appended to V (row 64 of the PV output);
  normalization is DVE reciprocal + gpsimd partition_broadcast + one DVE
  multiply that also casts attn-out to bf16.
- FFN SiLU is computed as x*0.5*(1+tanh(x/2)) (exact), with the 0.5 folded
  into w2 host-side, so every table-based activation in the kernel (exp,
  tanh, square, copy, identity) lives in the same activation table set and
  only the three tiny rmsnorm Sqrt ops force a table switch.
- All bulk DMAs are batched and issued from the gpsimd queue.
"""

import sys

sys.path.insert(0, "/opt/trn_rl_repo")

import numpy as np
import ml_dtypes

import concourse.bass as bass
import concourse.mybir as mybir
import concourse.tile as tile
from concourse import bacc
from concourse.bass_utils import run_bass_kernel_spmd
from concourse.masks import make_identity

BF = ml_dtypes.bfloat16
FP32 = mybir.dt.float32
BF16 = mybir.dt.bfloat16
AF = mybir.ActivationFunctionType
ALU = mybir.AluOpType

B, L, D, H, HD, F = 4, 2048, 1024, 16, 64, 4096
MASK_VAL = -30000.0

# C0 pass: q cols [0, 512), key tiles 0..7, score width W0 = 512 - j0*128
KTI0 = 8
W0 = [512 - (k // 2) * 128 for k in range(KTI0)]
CUM0 = [sum(W0[:k]) for k in range(KTI0)]
TOT0 = sum(W0)                       # 2560
# C1 pass: q cols [512, 1024), key tiles 0..15
KTI1 = 16
C1S = [max(512, (k // 2) * 128) for k in range(KTI1)]   # global col start
W1 = [1024 - c for c in C1S]
CUM1 = [sum(W1[:k]) for k in range(KTI1)]
TOT1 = sum(W1)                       # 6656
BIAS_PER_HEAD = TOT0 + TOT1          # 9216
BIAS_TOT = H * 128 * BIAS_PER_HEAD


def build_nc():
    nc = bacc.Bacc(None, target_bir_lowering=False)

    xt = nc.dram_tensor("xt", [D, L], BF16, kind="ExternalInput")
    xq = nc.dram_tensor("xq", [D, 1024], BF16, kind="ExternalInput")
    xres = nc.dram_tensor("xres", [D, 1024], FP32, kind="ExternalInput")
    biast = nc.dram_tensor("biast", [BIAS_TOT], BF16, kind="ExternalInput")
    wqt = nc.dram_tensor("wqt", [D, D], BF16, kind="ExternalInput")
    wkt = nc.dram_tensor("wkt", [D, D], BF16, kind="ExternalInput")
    wvt = nc.dram_tensor("wvt", [D, D], BF16, kind="ExternalInput")
    wot = nc.dram_tensor("wot", [D, D], BF16, kind="ExternalInput")
    w1t = nc.dram_tensor("w1t", [D, F], BF16, kind="ExternalInput")
    w2t = nc.dram_tensor("w2t", [F, D], BF16, kind="ExternalInput")
    bq = nc.dram_tensor("bq", [D], FP32, kind="ExternalInput")
    bk = nc.dram_tensor("bk", [D], FP32, kind="ExternalInput")
    bv = nc.dram_tensor("bv", [D], FP32, kind="ExternalInput")
    bo = nc.dram_tensor("bo", [D], FP32, kind="ExternalInput")
    b1h = nc.dram_tensor("b1h", [F], FP32, kind="ExternalInput")  # 0.5*b1
    b1f = nc.dram_tensor("b1f", [F], FP32, kind="ExternalInput")  # b1
    b2 = nc.dram_tensor("b2", [D], FP32, kind="ExternalInput")
    n1w = nc.dram_tensor("n1w", [D], FP32, kind="ExternalInput")
    n2w = nc.dram_tensor("n2w", [D], FP32, kind="ExternalInput")
    outp = nc.dram_tensor("outp", [1024, D], FP32, kind="ExternalOutput")

    def emit(tc):
        # ---------------- constants ------------------------------------
        pconst = tc.alloc_tile_pool(name="pconst", bufs=1, side="left")
        ones1 = pconst.tile([128, 1], BF16)
        nc.vector.memset(ones1[:], 1.0)
        identB = pconst.tile([128, 128], BF16)
        make_identity(nc, identB[:])
        identT = pconst.tile([128, 128], FP32)
        make_identity(nc, identT[:])
        bqt = pconst.tile([128, 8], FP32)
        nc.sync.dma_start(bqt[:], bq.rearrange("(g p) -> p g", p=128))
        bkt = pconst.tile([128, 8], FP32)
        nc.sync.dma_start(bkt[:], bk.rearrange("(g p) -> p g", p=128))
        bot = pconst.tile([128, 8], FP32)
        nc.sync.dma_start(bot[:], bo.rearrange("(g p) -> p g", p=128))
        b1ht = pconst.tile([128, 32], FP32)
        nc.sync.dma_start(b1ht[:], b1h.rearrange("(g p) -> p g", p=128))
        b1ft = pconst.tile([128, 32], FP32)
        nc.sync.dma_start(b1ft[:], b1f.rearrange("(g p) -> p g", p=128))
        b2t = pconst.tile([128, 8], FP32)
        nc.sync.dma_start(b2t[:], b2.rearrange("(g p) -> p g", p=128))
        n1t = pconst.tile([128, 8], FP32)
        nc.sync.dma_start(n1t[:], n1w.rearrange("(g p) -> p g", p=128))
        n2t = pconst.tile([128, 8], FP32)
        nc.sync.dma_start(n2t[:], n2w.rearrange("(g p) -> p g", p=128))
        epsb = pconst.tile([1, 1], FP32)
        nc.vector.memset(epsb[:], 1e-6)
        bvrow = pconst.tile([1, 1024], FP32)
        nc.sync.dma_start(bvrow[:], bv[None, :])
        bvB = pconst.tile([128, 1024], FP32)
        nc.gpsimd.partition_broadcast(bvB[:], bvrow[:])

        # ---------------- Phase A: norm1 -------------------------------
        pht = tc.alloc_tile_pool(name="pht", bufs=1, side="right")
        ht = pht.tile([128, 8, L], BF16)     # rmsnorm(x)^T, full batch
        hq = pht.tile([128, 8, 1024], BF16)  # rmsnorm(x)^T, q-cols only

        with tc.tile_pool(name="pxs", bufs=1) as pxs, \
             tc.tile_pool(name="psq", bufs=2) as psq, \
             tc.tile_pool(name="pnrm", bufs=1) as pnrm, \
             tc.tile_pool(name="ppsA", bufs=1, space="PSUM") as ppsA:
            xs = pxs.tile([128, 8, L], BF16)
            xqs = pxs.tile([128, 8, 1024], BF16)
            ssq = ppsA.tile([1, L], FP32)
            ssqq = ppsA.tile([1, 1024], FP32)
            for c in range(8):
                nc.sync.dma_start(xs[:, c], xt[c * 128:(c + 1) * 128, :])
                sq = psq.tile([128, L], BF16, tag="sq")
                nc.scalar.activation(sq[:], xs[:, c], AF.Square)
                for s in range(0, L, 512):
                    nc.tensor.matmul(ssq[:, s:s + 512], ones1[:],
                                     sq[:, s:s + 512],
                                     start=(c == 0), stop=(c == 7))
                nc.sync.dma_start(xqs[:, c], xq[c * 128:(c + 1) * 128, :])
                sqq = psq.tile([128, 1024], BF16, tag="sqq")
                nc.scalar.activation(sqq[:], xqs[:, c], AF.Square)
                for s in range(0, 1024, 512):
                    nc.tensor.matmul(ssqq[:, s:s + 512], ones1[:],
                                     sqq[:, s:s + 512],
                                     start=(c == 0), stop=(c == 7))
            rms = pnrm.tile([1, L], FP32)
            nc.scalar.activation(rms[:], ssq[:], AF.Sqrt,
                                 scale=1.0 / D, bias=epsb[:])
            rinv = pnrm.tile([1, L], FP32)
            nc.vector.reciprocal(rinv[:], rms[:])
            rB = pnrm.tile([128, L], FP32)
            nc.gpsimd.partition_broadcast(rB[:], rinv[:])
            rmsq = pnrm.tile([1, 1024], FP32)
            nc.scalar.activation(rmsq[:], ssqq[:], AF.Sqrt,
                                 scale=1.0 / D, bias=epsb[:])
            rinvq = pnrm.tile([1, 1024], FP32)
            nc.vector.reciprocal(rinvq[:], rmsq[:])
            rBq = pnrm.tile([128, 1024], FP32)
            nc.gpsimd.partition_broadcast(rBq[:], rinvq[:])
            for c in range(8):
                nc.vector.scalar_tensor_tensor(
                    ht[:, c, :], xs[:, c], n1t[:, c:c + 1], rB[:],
                    op0=ALU.mult, op1=ALU.mult)
                nc.vector.scalar_tensor_tensor(
                    hq[:, c, :], xqs[:, c], n1t[:, c:c + 1], rBq[:],
                    op0=ALU.mult, op1=ALU.mult)

        # ---------------- Phase B: Q, K (feature-major), V (row-major) --
        pkv = tc.alloc_tile_pool(name="pkv", bufs=1, side="left")
        kt = pkv.tile([128, 8, L], BF16)           # K^T [feat, key]
        qt = pkv.tile([128, 8, 1024], BF16)        # Q^T [feat, query]
        vv = pkv.tile([128, 16, 16, 65], BF16)     # V rows [key, (h, hd+1)]
        nc.vector.memset(vv[:, :, :, 64:65], 1.0)

        with tc.tile_pool(name="pw2", bufs=3) as pw2, \
             tc.tile_pool(name="pwv", bufs=1) as pwv, \
             tc.tile_pool(name="pps2", bufs=4, space="PSUM") as pps2:
            for g in range(8):
                wkg = pw2.tile([128, 8, 128], BF16, tag="wkg")
                nc.sync.dma_start(
                    wkg[:], wkt[:, g * 128:(g + 1) * 128]
                    .rearrange("(c p) o -> p c o", p=128))
                for s in range(0, L, 512):
                    pk = pps2.tile([128, 512], FP32, tag="pp", name="pk")
                    for c in range(8):
                        nc.tensor.matmul(pk[:], wkg[:, c], ht[:, c, s:s + 512],
                                         start=(c == 0), stop=(c == 7))
                    nc.scalar.activation(kt[:, g, s:s + 512], pk[:],
                                         AF.Identity, bias=bkt[:, g:g + 1])
                wqg = pw2.tile([128, 8, 128], BF16, tag="wqg")
                nc.sync.dma_start(
                    wqg[:], wqt[:, g * 128:(g + 1) * 128]
                    .rearrange("(c p) o -> p c o", p=128))
                for s in range(0, 1024, 512):
                    pq = pps2.tile([128, 512], FP32, tag="pp", name="pq")
                    for c in range(8):
                        nc.tensor.matmul(pq[:], wqg[:, c], hq[:, c, s:s + 512],
                                         start=(c == 0), stop=(c == 7))
                    nc.scalar.activation(qt[:, g, s:s + 512], pq[:],
                                         AF.Identity, bias=bqt[:, g:g + 1])
            wvs = pwv.tile([128, 8, 1024], BF16, tag="wvs")
            for c in range(8):
                nc.sync.dma_start(wvs[:, c], wvt[c * 128:(c + 1) * 128, :])
            for lt in range(16):
                for hf in range(2):
                    pv = pps2.tile([128, 512], FP32, tag="pp", name="pv")
                    for c in range(8):
                        nc.tensor.matmul(
                            pv[:], ht[:, c, lt * 128:(lt + 1) * 128],
                            wvs[:, c, hf * 512:(hf + 1) * 512],
                            start=(c == 0), stop=(c == 7))
                    nc.vector.tensor_add(
                        vv[:, lt, hf * 8:(hf + 1) * 8, 0:64],
                        pv[:].rearrange("p (h e) -> p h e", e=64),
                        bvB[:, hf * 512:(hf + 1) * 512]
                        .rearrange("p (h e) -> p h e", e=64))

        pht.release()

        # ---------------- Phase C: attention ----------------------------
        pao = tc.alloc_tile_pool(name="pao", bufs=1, side="right")
        aoT = pao.tile([128, 8, 1024], BF16)   # attn-out^T [feat, query]

        def attn_pass(npass, kti_n, wlist, cumlist, bias_off):
            sfx = str(npass)
            pP = tc.alloc_tile_pool(name="pP" + sfx, bufs=1, side="right")
            P = pP.tile([128, kti_n, 512], BF16, name="P" + sfx)
            # zero causally-dead columns once (PV reads them as zeros)
            for k in range(kti_n):
                dead = 512 - wlist[k]
                if dead > 0:
                    nc.vector.memset(P[:, k, 0:dead], 0.0)
            qbase = npass * 512
            tot = cumlist[-1] + wlist[-1]
            with tc.tile_pool(name="pbias" + sfx, bufs=2) as pbias, \
                 tc.tile_pool(name="pz" + sfx, bufs=2) as pz, \
                 tc.tile_pool(name="pS" + sfx, bufs=3, space="PSUM") as pS, \
                 tc.tile_pool(name="pPo" + sfx, bufs=2, space="PSUM") as pPo:
                for h in range(H):
                    hg, hp = h // 2, 64 * (h % 2)
                    bst = pbias.tile([128, tot], BF16, tag="bst",
                                     name="bst" + sfx)
                    off = bias_off + h * 128 * tot
                    nc.sync.dma_start(
                        bst[:, :tot],
                        biast[off:off + 128 * tot]
                        .rearrange("(p w) -> p w", p=128))
                    pout = pPo.tile([65, 512], FP32, tag="po", name="po" + sfx)
                    for k in range(kti_n):
                        w = wlist[k]
                        dead = 512 - w
                        S = pS.tile([128, 512], FP32, tag="S", name="S" + sfx)
                        nc.tensor.matmul(
                            S[:, 0:w],
                            kt[hp:hp + 64, hg, k * 128:(k + 1) * 128],
                            qt[hp:hp + 64, hg, qbase + dead:qbase + 512],
                            start=True, stop=False)
                        nc.tensor.matmul(
                            S[:, 0:w], identB[:],
                            bst[:, cumlist[k]:cumlist[k] + w],
                            start=False, stop=True)
                        nc.scalar.activation(P[:, k, dead:512], S[:, 0:w],
                                             AF.Exp)
                        nc.tensor.matmul(pout[:], vv[:, k, h], P[:, k, :],
                                         start=(k == 0), stop=(k == kti_n - 1))
                    zrec = pz.tile([1, 512], FP32, tag="zr", name="zr" + sfx)
                    nc.vector.reciprocal(zrec[:], pout[64:65, :])
                    zbB = pz.tile([64, 512], FP32, tag="zb", name="zb" + sfx)
                    nc.gpsimd.partition_broadcast(zbB[:], zrec[:])
                    nc.vector.tensor_mul(
                        aoT[hp:hp + 64, hg, qbase:qbase + 512],
                        pout[0:64, :], zbB[:])
            pP.release()

        attn_pass(0, KTI0, W0, CUM0, 0)
        attn_pass(1, KTI1, W1, CUM1, H * 128 * TOT0)

        pkv.release()

        # ---------------- Phase D: W_O + FFN, per q-half ----------------
        pxr = tc.alloc_tile_pool(name="pxr", bufs=1, side="left")
        xr = pxr.tile([128, 8, 1024], FP32)
        for c in range(8):
            nc.sync.dma_start(xr[:, c], xres[c * 128:(c + 1) * 128, :])
        pwo = tc.alloc_tile_pool(name="pwo", bufs=1, side="left")
        wov = pwo.tile([128, 8, 8, 128], BF16)   # all of wo resident
        for g in range(8):
            nc.sync.dma_start(
                wov[:, g], wot[:, g * 128:(g + 1) * 128]
                .rearrange("(c p) o -> p c o", p=128))
        px2 = tc.alloc_tile_pool(name="px2", bufs=1, side="left")
        x2T = px2.tile([128, 8, 1024], FP32)

        def ffn_half(n):
            sfx = "h" + str(n)
            q0 = n * 512
            with tc.tile_pool(name="ph2" + sfx, bufs=1) as ph2, \
                 tc.tile_pool(name="pf1" + sfx, bufs=1) as pf1, \
                 tc.tile_pool(name="pwf" + sfx, bufs=3) as pwf, \
                 tc.tile_pool(name="pnrm" + sfx, bufs=1) as pnrm, \
                 tc.tile_pool(name="psq" + sfx, bufs=2) as psq, \
                 tc.tile_pool(name="pu" + sfx, bufs=3) as pu, \
                 tc.tile_pool(name="pot" + sfx, bufs=2) as pot, \
                 tc.tile_pool(name="ppsD" + sfx, bufs=4, space="PSUM") as ppsD, \
                 tc.tile_pool(name="ppsS" + sfx, bufs=1, space="PSUM") as ppsS:
                # W_O + residual
                for g in range(8):
                    po = ppsD.tile([128, 512], FP32, tag="pp", name="po" + sfx)
                    for c in range(8):
                        nc.tensor.matmul(po[:], wov[:, g, c],
                                         aoT[:, c, q0:q0 + 512],
                                         start=(c == 0), stop=(c == 7))
                    nc.vector.scalar_tensor_tensor(
                        x2T[:, g, q0:q0 + 512], po[:], bot[:, g:g + 1],
                        xr[:, g, q0:q0 + 512], op0=ALU.add, op1=ALU.add)
                # norm2
                ssq = ppsS.tile([1, 512], FP32, name="ssq" + sfx)
                for c in range(8):
                    sq = psq.tile([128, 512], BF16, tag="sq", name="sq" + sfx)
                    nc.scalar.activation(sq[:], x2T[:, c, q0:q0 + 512],
                                         AF.Square)
                    nc.tensor.matmul(ssq[:], ones1[:], sq[:],
                                     start=(c == 0), stop=(c == 7))
                rms = pnrm.tile([1, 512], FP32, name="rms" + sfx)
                nc.scalar.activation(rms[:], ssq[:], AF.Sqrt,
                                     scale=1.0 / D, bias=epsb[:])
                rinv = pnrm.tile([1, 512], FP32, name="rinv" + sfx)
                nc.vector.reciprocal(rinv[:], rms[:])
                rB = pnrm.tile([128, 512], FP32, name="rB" + sfx)
                nc.gpsimd.partition_broadcast(rB[:], rinv[:])
                h2 = ph2.tile([128, 8, 512], BF16, name="h2" + sfx)
                for c in range(8):
                    nc.vector.scalar_tensor_tensor(
                        h2[:, c, :], x2T[:, c, q0:q0 + 512], n2t[:, c:c + 1],
                        rB[:], op0=ALU.mult, op1=ALU.mult)
                # FFN1: f1 = (z + b1) * (1 + tanh(z/2 + b1/2)) = 2*silu(z+b1)
                f1T = pf1.tile([128, 32, 512], BF16, name="f1T" + sfx)
                for gf in range(32):
                    w1g = pwf.tile([128, 8, 128], BF16, tag="w1g",
                                   name="w1g" + sfx)
                    nc.sync.dma_start(
                        w1g[:], w1t[:, gf * 128:(gf + 1) * 128]
                        .rearrange("(c p) o -> p c o", p=128))
                    z = ppsD.tile([128, 512], FP32, tag="pp", name="z" + sfx)
                    for c in range(8):
                        nc.tensor.matmul(z[:], w1g[:, c], h2[:, c, :],
                                         start=(c == 0), stop=(c == 7))
                    u = pu.tile([128, 512], BF16, tag="u", name="u" + sfx)
                    nc.scalar.activation(u[:], z[:], AF.Tanh,
                                         scale=0.5, bias=b1ht[:, gf:gf + 1])
                    t2 = pu.tile([128, 512], BF16, tag="t2", name="t2" + sfx)
                    nc.vector.scalar_tensor_tensor(
                        t2[:], z[:], b1ft[:, gf:gf + 1], u[:],
                        op0=ALU.add, op1=ALU.mult)
                    nc.vector.scalar_tensor_tensor(
                        f1T[:, gf, :], z[:], b1ft[:, gf:gf + 1], t2[:],
                        op0=ALU.add, op1=ALU.add)
                # FFN2 + residual + transpose + store
                for g in range(8):
                    w2g = pwf.tile([128, 32, 128], BF16, tag="w2g",
                                   name="w2g" + sfx)
                    nc.sync.dma_start(
                        w2g[:], w2t[:, g * 128:(g + 1) * 128]
                        .rearrange("(c p) o -> p c o", p=128))
                    o = ppsD.tile([128, 512], FP32, tag="pp", name="o" + sfx)
                    for cf in range(32):
                        nc.tensor.matmul(o[:], w2g[:, cf], f1T[:, cf, :],
                                         start=(cf == 0), stop=(cf == 31))
                    og = pot.tile([128, 512], FP32, tag="og", name="og" + sfx)
                    nc.vector.scalar_tensor_tensor(
                        og[:], o[:], b2t[:, g:g + 1],
                        x2T[:, g, q0:q0 + 512], op0=ALU.add, op1=ALU.add)
                    for j in range(4):
                        pt = ppsD.tile([128, 128], FP32, tag="pt", bufs=2,
                                       name="pt" + sfx)
                        nc.tensor.transpose(pt[:], og[:, j * 128:(j + 1) * 128],
                                            identT[:])
                        ot = pot.tile([128, 128], FP32, tag="ot",
                                      name="ot" + sfx)
                        nc.vector.tensor_copy(ot[:], pt[:])
                        nc.sync.dma_start(
                            outp[(n * 4 + j) * 128:(n * 4 + j + 1) * 128,
                                 g * 128:(g + 1) * 128],
                            ot[:])

        ffn_half(0)
        ffn_half(1)

        pao.release()
        px2.release()
        pwo.release()
        pxr.release()
        pconst.release()

    with tile.TileContext(nc, pool_alloc_mode="queue") as tc:
        emit(tc)

    nc.compile()
    return nc


_NC = None


def _get_nc():
    global _NC
    if _NC is None:
        _NC = build_nc()
    return _NC


def _prep_core_inputs(c, x, rel_pos_bias, wq, bq, wk, bk, wv, bv, wo, bo,
                      norm1_w, norm2_w, w1, b1, w2, b2):
    b, par = c // 2, c % 2
    xb = x[b]                                   # [L, D] f32
    qrows = np.concatenate(
        [np.arange(128 * (2 * j + par), 128 * (2 * j + par) + 128)
         for j in range(8)])
    rel = np.asarray(rel_pos_bias[0], dtype=np.float32)   # [H, L, L]

    bias_flat = np.empty(BIAS_TOT, dtype=BF)
    # C0 region: per head, key tiles 0..7, q cols [dead, 512) of this core
    off = 0
    for npass, (kti_n, wlist, cumlist, qbase) in enumerate(
            [(KTI0, W0, CUM0, 0), (KTI1, W1, CUM1, 512)]):
        tot = cumlist[-1] + wlist[-1]
        blk = np.empty((H, 128, tot), dtype=np.float32)
        for k in range(kti_n):
            w = wlist[k]
            dead = 512 - w
            qcols = qbase + dead + np.arange(w)           # local q col idx
            j = qcols // 128                              # slot
            i = 2 * j + par                               # q block
            qglob = i * 128 + (qcols % 128)               # global q row
            k0 = k * 128
            sub = rel[:, qglob, k0:k0 + 128]              # [H, w, 128]
            mask = (k0 + np.arange(128))[None, :] > qglob[:, None]  # [w,128]
            sub = np.where(mask[None], MASK_VAL, sub)
            blk[:, :, cumlist[k]:cumlist[k] + w] = sub.transpose(0, 2, 1)
        bias_flat[off:off + H * 128 * tot] = blk.reshape(-1).astype(BF)
        off += H * 128 * tot

    scale = HD ** -0.5
    return {
        "xt": np.ascontiguousarray(xb.T).astype(BF),
        "xq": np.ascontiguousarray(xb[qrows].T).astype(BF),
        "xres": np.ascontiguousarray(xb[qrows].T).astype(np.float32),
        "biast": bias_flat,
        "wqt": np.ascontiguousarray(wq.T * scale).astype(BF),
        "wkt": np.ascontiguousarray(wk.T).astype(BF),
        "wvt": np.ascontiguousarray(wv.T).astype(BF),
        "wot": np.ascontiguousarray(wo.T).astype(BF),
        "w1t": np.ascontiguousarray(w1.T).astype(BF),
        "w2t": np.ascontiguousarray(w2.T * 0.5).astype(BF),
        "bq": (bq * scale).astype(np.float32),
        "bk": bk.astype(np.float32),
        "bv": bv.astype(np.float32), "bo": bo.astype(np.float32),
        "b1h": (0.5 * b1).astype(np.float32),
        "b1f": b1.astype(np.float32),
        "b2": b2.astype(np.float32),
        "n1w": norm1_w.astype(np.float32), "n2w": norm2_w.astype(np.float32),
    }


def kernel(**inputs):
    inputs = {k: np.asarray(v) for k, v in inputs.items()}
    nc = _get_nc()
    in_maps = [_prep_core_inputs(c, **inputs) for c in range(8)]
    res = run_bass_kernel_spmd(nc, in_maps, core_ids=list(range(8)))
    out = np.empty((B, L, D), dtype=np.float32)
    for c in range(8):
        b, par = c // 2, c % 2
        o = res.results[c]["outp"]
        for j in range(8):
            i = 2 * j + par
            out[b, 128 * i:128 * i + 128] = o[128 * j:128 * j + 128]
    return out
